# revision 10
# baseline (speedup 1.0000x reference)
"""GRU kernel for Trainium2, 8 NeuronCores, data-parallel over batch.

Strategy
--------
reference:  per step t (T=512):
    gi = [h, x_t]; r = sig(gi@Wr+br); z = sig(gi@Wz+bz)
    hh = tanh([h*r, x_t]@Wl+bl); h = (1-z)h + z*hh; out_t = relu(h@Wo+bo)

Decomposition per core (B_local=8 rows):
  Phase 1 (parallel over all t): XgT = Wx_g^T @ x^T + b_g for g in {r,z,l}
     (f32r matmuls, N=512) -> DRAM, transposed layout [H, B_local*T].
  Recurrence (serial, fully transposed domain; state hT [128 part, 8 chunks*8b]):
     per step: ar^T/az^T = Wh^T h^T (bf16 stationary weights resident in SBUF,
     LDW+MM pairs at ~33ns), + X slice, sigmoid; rh^T = r^T*h^T;
     al^T likewise from rh^T; h_new^T elementwise. h history accumulates in
     SBUF as bf16 and every 16 steps the output projection
     outT = relu(Wo^T hT + bo) runs fused (bf16), written blockwise to DRAM.
  Host: pre-transposes x per core, un-permutes outT blocks.
"""
import os
import numpy as np
from contextlib import ExitStack, nullcontext

import concourse.bass as bass
import concourse.tile as tile
from concourse import bacc, mybir
from concourse import bass_utils

B, T_FULL, D, H = 64, 512, 1024, 1024
NCORES = 8
BL = B // NCORES            # 8 batch rows per core
KC = H // 128               # 8 contraction chunks
JC = H // 128               # 8 output chunks
BLK = 16                    # recurrence steps per output-projection block

f32 = mybir.dt.float32
f32r = mybir.dt.float32r
bf16 = mybir.dt.bfloat16
f8 = mybir.dt.float8e3
AF = mybir.ActivationFunctionType
ALU = mybir.AluOpType

# Recurrent weights are held as fp8 e3m4, scaled into its normal range
# ([0.25, 15.5]); FWL then loads them at 4 elem/cycle (vs 2 for bf16),
# halving the LDWEIGHTS-bound recurrence.  max|Wh|=1/sqrt(2048)=0.0221
# -> x512 = 11.3; max|Wo|=1/32 -> x256 = 8.  The inverse scale is folded
# into the existing gate-add (scalar_tensor_tensor) / activation ops.
WH_SCALE = 512.0
WO_SCALE = 256.0

_CACHE = {}


def build_program(T, repeat=1):
    cols = BL * T           # columns of the transposed activations
    nblk = T // BLK
    assert T % BLK == 0

    nc = bacc.Bacc("TRN2", target_bir_lowering=False, debug=False, num_devices=1)

    xT = nc.dram_tensor("xT", (H, cols), f32, kind="ExternalInput").ap()
    wx = {g: nc.dram_tensor(f"wx{g}", (D, H), f32, kind="ExternalInput").ap()
          for g in "rzl"}
    wh = {g: nc.dram_tensor(f"wh{g}", (H, H), f32, kind="ExternalInput").ap()
          for g in "rzl"}
    bias = {g: nc.dram_tensor(f"b{g}", (H, 1), f32, kind="ExternalInput").ap()
            for g in "rzl"}
    wo_d = nc.dram_tensor("wo", (H, H), f32, kind="ExternalInput").ap()
    bo_d = nc.dram_tensor("bo", (H, 1), f32, kind="ExternalInput").ap()
    outT = nc.dram_tensor("outT", (128, nblk * JC * BLK * BL), f32,
                          kind="ExternalOutput").ap()

    with tile.TileContext(nc) as tc, ExitStack() as top:
        dram = top.enter_context(tc.tile_pool(name="dram", bufs=1, space="DRAM"))
        xg_d = {g: dram.tile([H, cols], f32, tag=f"X{g}", name=f"X{g}") for g in "rzl"}

        # repeat>1 wraps the whole body in a hardware loop — a timing-only
        # build that amplifies device time over the fixed RPC overhead.
        loop = tc.For_i(0, repeat, 1) if repeat > 1 else nullcontext()
        top.enter_context(loop)

        # ---------------- Phase 1: x projections (f32r) ----------------
        with ExitStack() as ctx:
            wp = ctx.enter_context(tc.tile_pool(name="p1w", bufs=1))
            xp = ctx.enter_context(tc.tile_pool(name="p1x", bufs=2))
            pp = ctx.enter_context(tc.tile_pool(name="p1ps", bufs=4, space="PSUM"))
            op = ctx.enter_context(tc.tile_pool(name="p1o", bufs=3))
            bp = ctx.enter_context(tc.tile_pool(name="p1b", bufs=1))

            wx_sb = {}
            bt = {}
            for g in "rzl":
                wx_sb[g] = wp.tile([128, KC * H], f32r, tag=f"wx{g}", name=f"wx{g}sb")
                for kc in range(KC):
                    nc.sync.dma_start(
                        wx_sb[g][:, kc * H:(kc + 1) * H],
                        wx[g][kc * 128:(kc + 1) * 128, :].bitcast(f32r))
                bt[g] = bp.tile([128, JC], f32, tag=f"b{g}", name=f"bt{g}")
                for jc in range(JC):
                    nc.sync.dma_start(bt[g][:, jc:jc + 1],
                                      bias[g][jc * 128:(jc + 1) * 128, :])

            NCB = 512
            for cb in range(cols // NCB):
                xt = xp.tile([128, KC * NCB], f32r, tag="xt")
                for kc in range(KC):
                    nc.sync.dma_start(
                        xt[:, kc * NCB:(kc + 1) * NCB],
                        xT[kc * 128:(kc + 1) * 128,
                           cb * NCB:(cb + 1) * NCB].bitcast(f32r))
                for g in "rzl":
                    for jc in range(JC):
                        ps = pp.tile([128, NCB], f32, tag="ps")
                        for kc in range(KC):
                            nc.tensor.matmul(
                                ps[:],
                                lhsT=wx_sb[g][:, kc * H + jc * 128:
                                              kc * H + (jc + 1) * 128],
                                rhs=xt[:, kc * NCB:(kc + 1) * NCB],
                                start=(kc == 0), stop=(kc == KC - 1))
                        ot = op.tile([128, NCB], f32, tag="ot")
                        nc.scalar.activation(ot[:], ps[:], AF.Identity,
                                             bias=bt[g][:, jc:jc + 1])
                        nc.sync.dma_start(
                            xg_d[g][jc * 128:(jc + 1) * 128,
                                    cb * NCB:(cb + 1) * NCB], ot[:])

        # Phase-1 writes X* to DRAM via DMA; DRAM-tile RAW deps are not
        # reliably tracked by the scheduler, so fence before consuming.
        tc.strict_bb_all_engine_barrier()

        # ------------- Recurrence + fused output projection -------------
        with ExitStack() as ctx:
            wp = ctx.enter_context(tc.tile_pool(name="rw", bufs=1))
            sg = ctx.enter_context(tc.tile_pool(name="stg", bufs=2))
            xb = ctx.enter_context(tc.tile_pool(name="xblk", bufs=2))
            hi = ctx.enter_context(tc.tile_pool(name="hist", bufs=2))
            st = ctx.enter_context(tc.tile_pool(name="state", bufs=2))
            el = ctx.enter_context(tc.tile_pool(name="elt", bufs=2))
            pg = ctx.enter_context(tc.tile_pool(name="psg", bufs=2, space="PSUM"))
            p3 = ctx.enter_context(tc.tile_pool(name="ps3", bufs=2, space="PSUM"))
            o3 = ctx.enter_context(tc.tile_pool(name="o3", bufs=3))
            bp = ctx.enter_context(tc.tile_pool(name="rb", bufs=1))

            # resident fp8 weights (staged through f32, scaled into e3m4 range)
            wh_sb = {}
            for g in "rzl":
                wh_sb[g] = wp.tile([128, KC * H], f8, tag=f"wh{g}", name=f"wh{g}sb")
                for kc in range(KC):
                    stg = sg.tile([128, H], f32, tag="stg")
                    nc.sync.dma_start(stg[:], wh[g][kc * 128:(kc + 1) * 128, :])
                    nc.vector.tensor_scalar_mul(
                        wh_sb[g][:, kc * H:(kc + 1) * H], stg[:], WH_SCALE)
            wo_sb = wp.tile([128, KC * H], f8, tag="wo")
            for kc in range(KC):
                stg = sg.tile([128, H], f32, tag="stg")
                nc.sync.dma_start(stg[:], wo_d[kc * 128:(kc + 1) * 128, :])
                nc.vector.tensor_scalar_mul(
                    wo_sb[:, kc * H:(kc + 1) * H], stg[:], WO_SCALE)
            bo_t = bp.tile([128, JC], f32, tag="bo")
            for jc in range(JC):
                nc.sync.dma_start(bo_t[:, jc:jc + 1],
                                  bo_d[jc * 128:(jc + 1) * 128, :])

            CW = BL * KC        # 64: columns of a state tile (chunk-major, b minor)
            hT = st.tile([128, CW], f32, tag="hT")
            nc.vector.memset(hT[:], 0.0)
            hz = bp.tile([128, CW], bf16, tag="h0")
            nc.vector.memset(hz[:], 0.0)
            hprev_src, hprev_off = hz, 0       # bf16 h^T of previous step

            def gate_mm(ps, wt, src, off):
                for jc in range(JC):
                    for kc in range(KC):
                        nc.tensor.matmul(
                            ps[:, jc * BL:(jc + 1) * BL],
                            lhsT=wt[:, (kc * JC + jc) * 128:
                                    (kc * JC + jc + 1) * 128],
                            rhs=src[:, off + kc * BL:off + (kc + 1) * BL],
                            start=(kc == 0), stop=(kc == KC - 1))

            for bi in range(nblk):
                xblk = {}
                for g in "rzl":
                    xblk[g] = xb.tile([128, KC * BLK * BL], f32, tag=f"xb{g}", name=f"xb{g}t")
                    for kc in range(KC):
                        nc.sync.dma_start(
                            xblk[g][:, kc * BLK * BL:(kc + 1) * BLK * BL],
                            xg_d[g][kc * 128:(kc + 1) * 128,
                                    bi * BLK * BL:(bi + 1) * BLK * BL])
                hist = hi.tile([128, BLK * CW], bf16, tag="hist")

                for dt in range(BLK):
                    def xsl(g):
                        return (xblk[g][:].rearrange("p (c s) -> p c s", c=KC)
                                [:, :, dt * BL:(dt + 1) * BL])
                    psr = pg.tile([128, CW], f32, tag="gr")
                    gate_mm(psr, wh_sb["r"], hprev_src, hprev_off)
                    psz = pg.tile([128, CW], f32, tag="gz")
                    gate_mm(psz, wh_sb["z"], hprev_src, hprev_off)

                    c3 = "p (c b) -> p c b"
                    tr = el.tile([128, CW], f32, tag="tr")
                    nc.vector.scalar_tensor_tensor(
                        tr[:].rearrange(c3, c=KC),
                        psr[:].rearrange(c3, c=KC), 1.0 / WH_SCALE, xsl("r"),
                        op0=ALU.mult, op1=ALU.add)
                    r = el.tile([128, CW], f32, tag="r")
                    nc.scalar.activation(r[:], tr[:], AF.Sigmoid)
                    rh = el.tile([128, CW], bf16, tag="rh")
                    nc.vector.tensor_mul(rh[:], r[:], hT[:])

                    psl = pg.tile([128, CW], f32, tag="gl")
                    gate_mm(psl, wh_sb["l"], rh, 0)

                    tz = el.tile([128, CW], f32, tag="tz")
                    nc.vector.scalar_tensor_tensor(
                        tz[:].rearrange(c3, c=KC),
                        psz[:].rearrange(c3, c=KC), 1.0 / WH_SCALE, xsl("z"),
                        op0=ALU.mult, op1=ALU.add)
                    z = el.tile([128, CW], f32, tag="z")
                    nc.scalar.activation(z[:], tz[:], AF.Sigmoid)

                    tl = el.tile([128, CW], f32, tag="tl")
                    nc.vector.scalar_tensor_tensor(
                        tl[:].rearrange(c3, c=KC),
                        psl[:].rearrange(c3, c=KC), 1.0 / WH_SCALE, xsl("l"),
                        op0=ALU.mult, op1=ALU.add)
                    hh = el.tile([128, CW], f32, tag="hh")
                    nc.scalar.activation(hh[:], tl[:], AF.Tanh)

                    d = el.tile([128, CW], f32, tag="d")
                    nc.vector.tensor_sub(d[:], hh[:], hT[:])
                    e = el.tile([128, CW], f32, tag="e")
                    nc.vector.tensor_mul(e[:], z[:], d[:])
                    hTn = st.tile([128, CW], f32, tag="hT")
                    nc.vector.tensor_add(hTn[:], hT[:], e[:])
                    nc.vector.tensor_copy(hist[:, dt * CW:(dt + 1) * CW], hTn[:])
                    hT = hTn
                    hprev_src, hprev_off = hist, dt * CW

                # fused output projection for this block (bf16).
                # Compact the strided (t, c, b) history view into contiguous
                # per-k-chunk rhs tiles first.
                hv = hist[:].rearrange("p (t c b) -> p t c b", t=BLK, c=KC)
                hcmp = o3.tile([128, KC * BLK * BL], bf16, tag="hcmp",
                               name="hcmp")
                for kc in range(KC):
                    nc.vector.tensor_copy(
                        hcmp[:, kc * BLK * BL:(kc + 1) * BLK * BL]
                        .rearrange("p (t b) -> p t b", t=BLK),
                        hv[:, :, kc, :])
                for jc in range(JC):
                    pso = p3.tile([128, BLK * BL], f32, tag="pso")
                    for kc in range(KC):
                        nc.tensor.matmul(
                            pso[:],
                            lhsT=wo_sb[:, (kc * JC + jc) * 128:
                                       (kc * JC + jc + 1) * 128],
                            rhs=hcmp[:, kc * BLK * BL:(kc + 1) * BLK * BL],
                            start=(kc == 0), stop=(kc == KC - 1))
                    ou = o3.tile([128, BLK * BL], f32, tag="ou")
                    nc.scalar.activation(ou[:], pso[:], AF.Relu,
                                         bias=bo_t[:, jc:jc + 1],
                                         scale=1.0 / WO_SCALE)
                    nc.sync.dma_start(
                        outT[:, (bi * JC + jc) * BLK * BL:
                             (bi * JC + jc + 1) * BLK * BL], ou[:])

        if repeat > 1:
            # WAR fence: next iteration's phase 1 rewrites X* in DRAM.
            tc.strict_bb_all_engine_barrier()

    nc.compile()
    return nc


def get_program(T, repeat=1):
    if (T, repeat) not in _CACHE:
        _CACHE[(T, repeat)] = build_program(T, repeat)
    return _CACHE[(T, repeat)]


def make_in_maps(input, Wr, br, Wz, bz, Wl, bl, Wo, bo):
    Tt = input.shape[1]
    cols = BL * Tt
    w_common = {
        "wxr": np.ascontiguousarray(Wr[H:]), "whr": np.ascontiguousarray(Wr[:H]),
        "wxz": np.ascontiguousarray(Wz[H:]), "whz": np.ascontiguousarray(Wz[:H]),
        "wxl": np.ascontiguousarray(Wl[H:]), "whl": np.ascontiguousarray(Wl[:H]),
        "br": np.ascontiguousarray(br.reshape(H, 1)),
        "bz": np.ascontiguousarray(bz.reshape(H, 1)),
        "bl": np.ascontiguousarray(bl.reshape(H, 1)),
        "wo": np.ascontiguousarray(Wo),
        "bo": np.ascontiguousarray(bo.reshape(H, 1)),
    }
    in_maps = []
    for c in range(NCORES):
        xl = np.asarray(input[c * BL:(c + 1) * BL], dtype=np.float32)
        xTl = np.ascontiguousarray(xl.transpose(2, 1, 0).reshape(H, cols))
        in_maps.append({"xT": xTl, **w_common})
    return in_maps


def assemble_output(results, Tt):
    nblk = Tt // BLK
    outs = []
    for c in range(NCORES):
        oT = results[c]["outT"]                  # [128, nblk*JC*BLK*BL]
        o = oT.reshape(128, nblk, JC, BLK, BL)   # p, bi, j, dt, b
        o = o.transpose(4, 1, 3, 2, 0).reshape(BL, Tt, H)
        outs.append(o)
    return np.ascontiguousarray(np.concatenate(outs, axis=0))


def kernel(input, Wr, br, Wz, bz, Wl, bl, Wo, bo):
    Tt = input.shape[1]
    prog = get_program(Tt)
    in_maps = make_in_maps(input, Wr, br, Wz, bz, Wl, bl, Wo, bo)
    res = bass_utils.run_bass_kernel_spmd(prog, in_maps,
                                          core_ids=list(range(NCORES)))
    return assemble_output(res.results, Tt)



# revision 23
# speedup vs baseline: 1.1351x; 1.1351x over previous
"""GRU kernel for Trainium2, 8 NeuronCores, data-parallel over batch.

Strategy
--------
reference:  per step t (T=512):
    gi = [h, x_t]; r = sig(gi@Wr+br); z = sig(gi@Wz+bz)
    hh = tanh([h*r, x_t]@Wl+bl); h = (1-z)h + z*hh; out_t = relu(h@Wo+bo)

Decomposition per core (B_local=8 rows):
  Phase 1 (parallel over all t): XgT = Wx_g^T @ x^T + b_g for g in {r,z,l}
     (f32r matmuls, N=512) -> DRAM, transposed layout [H, B_local*T].
  Recurrence (serial, fully transposed domain; state hT [128 part, 8 chunks*8b]):
     per step: ar^T/az^T = Wh^T h^T (bf16 stationary weights resident in SBUF,
     LDW+MM pairs at ~33ns), + X slice, sigmoid; rh^T = r^T*h^T;
     al^T likewise from rh^T; h_new^T elementwise. h history accumulates in
     SBUF as bf16 and every 16 steps the output projection
     outT = relu(Wo^T hT + bo) runs fused (bf16), written blockwise to DRAM.
  Host: pre-transposes x per core, un-permutes outT blocks.
"""
import os
import numpy as np
from contextlib import ExitStack, nullcontext

import concourse.bass as bass
import concourse.tile as tile
from concourse import bacc, mybir
from concourse import bass_utils

B, T_FULL, D, H = 64, 512, 1024, 1024
NCORES = 8
BL = B // NCORES            # 8 batch rows per core
KC = H // 128               # 8 contraction chunks
JC = H // 128               # 8 output chunks
BLK = 16                    # recurrence steps per output-projection block

f32 = mybir.dt.float32
f32r = mybir.dt.float32r
bf16 = mybir.dt.bfloat16
AF = mybir.ActivationFunctionType
ALU = mybir.AluOpType

_CACHE = {}


def build_program(T, repeat=1):
    cols = BL * T           # columns of the transposed activations
    nblk = T // BLK
    assert T % BLK == 0

    nc = bacc.Bacc("TRN2", target_bir_lowering=False, debug=False, num_devices=1)

    xT = nc.dram_tensor("xT", (H, cols), f32, kind="ExternalInput").ap()
    wx = {g: nc.dram_tensor(f"wx{g}", (D, H), f32, kind="ExternalInput").ap()
          for g in "rzl"}
    wh = {g: nc.dram_tensor(f"wh{g}", (H, H), f32, kind="ExternalInput").ap()
          for g in "rzl"}
    bias = {g: nc.dram_tensor(f"b{g}", (H, 1), f32, kind="ExternalInput").ap()
            for g in "rzl"}
    wo_d = nc.dram_tensor("wo", (H, H), f32, kind="ExternalInput").ap()
    bo_d = nc.dram_tensor("bo", (H, 1), f32, kind="ExternalInput").ap()
    outT = nc.dram_tensor("outT", (128, nblk * JC * BLK * BL), f32,
                          kind="ExternalOutput").ap()

    with tile.TileContext(nc) as tc, ExitStack() as top:
        dram = top.enter_context(tc.tile_pool(name="dram", bufs=1, space="DRAM"))
        xg_d = {g: dram.tile([H, cols], f32, tag=f"X{g}", name=f"X{g}") for g in "rzl"}

        # repeat>1 wraps the whole body in a hardware loop — a timing-only
        # build that amplifies device time over the fixed RPC overhead.
        loop = tc.For_i(0, repeat, 1) if repeat > 1 else nullcontext()
        top.enter_context(loop)

        # ---------------- Phase 1: x projections (f32r) ----------------
        with ExitStack() as ctx:
            wp = ctx.enter_context(tc.tile_pool(name="p1w", bufs=1))
            xp = ctx.enter_context(tc.tile_pool(name="p1x", bufs=2))
            pp = ctx.enter_context(tc.tile_pool(name="p1ps", bufs=4, space="PSUM"))
            op = ctx.enter_context(tc.tile_pool(name="p1o", bufs=3))
            bp = ctx.enter_context(tc.tile_pool(name="p1b", bufs=1))

            wx_sb = {}
            bt = {}
            for g in "rzl":
                wx_sb[g] = wp.tile([128, KC * H], f32r, tag=f"wx{g}", name=f"wx{g}sb")
                for kc in range(KC):
                    nc.sync.dma_start(
                        wx_sb[g][:, kc * H:(kc + 1) * H],
                        wx[g][kc * 128:(kc + 1) * 128, :].bitcast(f32r))
                bt[g] = bp.tile([128, JC], f32, tag=f"b{g}", name=f"bt{g}")
                for jc in range(JC):
                    nc.sync.dma_start(bt[g][:, jc:jc + 1],
                                      bias[g][jc * 128:(jc + 1) * 128, :])

            NCB = 512
            for cb in range(cols // NCB):
                xt = xp.tile([128, KC * NCB], f32r, tag="xt")
                for kc in range(KC):
                    nc.sync.dma_start(
                        xt[:, kc * NCB:(kc + 1) * NCB],
                        xT[kc * 128:(kc + 1) * 128,
                           cb * NCB:(cb + 1) * NCB].bitcast(f32r))
                for g in "rzl":
                    for jc in range(JC):
                        ps = pp.tile([128, NCB], f32, tag="ps")
                        for kc in range(KC):
                            nc.tensor.matmul(
                                ps[:],
                                lhsT=wx_sb[g][:, kc * H + jc * 128:
                                              kc * H + (jc + 1) * 128],
                                rhs=xt[:, kc * NCB:(kc + 1) * NCB],
                                start=(kc == 0), stop=(kc == KC - 1))
                        ot = op.tile([128, NCB], f32, tag="ot")
                        nc.scalar.activation(ot[:], ps[:], AF.Identity,
                                             bias=bt[g][:, jc:jc + 1])
                        nc.sync.dma_start(
                            xg_d[g][jc * 128:(jc + 1) * 128,
                                    cb * NCB:(cb + 1) * NCB], ot[:])

        # Phase-1 writes X* to DRAM via DMA; DRAM-tile RAW deps are not
        # reliably tracked by the scheduler, so fence before consuming.
        tc.strict_bb_all_engine_barrier()

        # ------------- Recurrence + fused output projection -------------
        with ExitStack() as ctx:
            wp = ctx.enter_context(tc.tile_pool(name="rw", bufs=1))
            sg = ctx.enter_context(tc.tile_pool(name="stg", bufs=2))
            xb = ctx.enter_context(tc.tile_pool(name="xblk", bufs=2))
            hi = ctx.enter_context(tc.tile_pool(name="hist", bufs=2))
            el = ctx.enter_context(tc.tile_pool(name="elt", bufs=2))
            pg = ctx.enter_context(tc.tile_pool(name="psg", bufs=2, space="PSUM"))
            p3 = ctx.enter_context(tc.tile_pool(name="ps3", bufs=2, space="PSUM"))
            o3 = ctx.enter_context(tc.tile_pool(name="o3", bufs=3))
            bp = ctx.enter_context(tc.tile_pool(name="rb", bufs=1))

            # resident bf16 weights (staged through f32)
            wh_sb = {}
            for g in "rzl":
                wh_sb[g] = wp.tile([128, KC * H], bf16, tag=f"wh{g}", name=f"wh{g}sb")
                for kc in range(KC):
                    stg = sg.tile([128, H], f32, tag="stg")
                    nc.sync.dma_start(stg[:], wh[g][kc * 128:(kc + 1) * 128, :])
                    nc.vector.tensor_copy(wh_sb[g][:, kc * H:(kc + 1) * H], stg[:])
            wo_sb = wp.tile([128, KC * H], bf16, tag="wo")
            for kc in range(KC):
                stg = sg.tile([128, H], f32, tag="stg")
                nc.sync.dma_start(stg[:], wo_d[kc * 128:(kc + 1) * 128, :])
                nc.vector.tensor_copy(wo_sb[:, kc * H:(kc + 1) * H], stg[:])
            bo_t = bp.tile([128, JC], f32, tag="bo")
            for jc in range(JC):
                nc.sync.dma_start(bo_t[:, jc:jc + 1],
                                  bo_d[jc * 128:(jc + 1) * 128, :])

            CW = BL * KC        # 64: columns of a state tile (chunk-major, b minor)
            HHALF = CW // 2     # split point for the h-update tail (k-chunks 0-3)
            st = ctx.enter_context(tc.tile_pool(name="state", bufs=2))
            hz = bp.tile([128, CW], bf16, tag="h0")
            nc.vector.memset(hz[:], 0.0)
            hT = st.tile([128, CW], f32, tag="hT")  # f32 master state
            nc.vector.memset(hT[:], 0.0)
            hprev_src, hprev_off = hz, 0       # bf16 h^T of previous step

            def gate_mm(ps, wt, src, off):
                for jc in range(JC):
                    for kc in range(KC):
                        nc.tensor.matmul(
                            ps[:, jc * BL:(jc + 1) * BL],
                            lhsT=wt[:, (kc * JC + jc) * 128:
                                    (kc * JC + jc + 1) * 128],
                            rhs=src[:, off + kc * BL:off + (kc + 1) * BL],
                            start=(kc == 0), stop=(kc == KC - 1))

            WO_PER_DT = (JC * KC) // BLK       # 4 Wo pairs interleaved per step

            def wo_flush(state):
                # Deferred relu+store of a finished Wo psum.  Emitted a step
                # later than its matmuls so the ACT FIFO never stalls on PE.
                if "done" in state:
                    pso, bo_blk, jc = state.pop("done")
                    ou = o3.tile([128, BLK * BL], f32, tag="ou")
                    nc.scalar.activation(ou[:], pso[:], AF.Relu,
                                         bias=bo_t[:, jc:jc + 1])
                    nc.sync.dma_start(
                        outT[:, (bo_blk * JC + jc) * BLK * BL:
                             (bo_blk * JC + jc + 1) * BLK * BL], ou[:])

            def wo_pairs(bo_blk, hv_prev, dt, state):
                # 4 lhsT/rhs pairs of the previous block's output projection,
                # placed in the psl->next-psr stall window.
                wo_flush(state)
                for i in range(WO_PER_DT):
                    pi = dt * WO_PER_DT + i
                    jc, kc = divmod(pi, KC)
                    if kc == 0:
                        state["pso"] = p3.tile([128, BLK * BL], f32,
                                               tag="pso", name="pso")
                    nc.tensor.matmul(
                        state["pso"][:],
                        lhsT=wo_sb[:, (kc * JC + jc) * 128:
                                   (kc * JC + jc + 1) * 128],
                        rhs=hv_prev[:, kc],
                        start=(kc == 0), stop=(kc == KC - 1))
                    if kc == KC - 1:
                        state["done"] = (state["pso"], bo_blk, jc)

            hist_prev = None
            wo_state = {}
            for bi in range(nblk):
                xblk = {}
                for g in "rzl":
                    xblk[g] = xb.tile([128, KC * BLK * BL], f32, tag=f"xb{g}", name=f"xb{g}t")
                    for kc in range(KC):
                        nc.sync.dma_start(
                            xblk[g][:, kc * BLK * BL:(kc + 1) * BLK * BL],
                            xg_d[g][kc * 128:(kc + 1) * 128,
                                    bi * BLK * BL:(bi + 1) * BLK * BL])
                hist = hi.tile([128, BLK * CW], bf16, tag="hist")
                hv_prev = (hist_prev[:].rearrange("p (t c b) -> p c t b",
                                                  t=BLK, c=KC)
                           if hist_prev is not None else None)

                for dt in range(BLK):
                    def xsl(g):
                        return (xblk[g][:].rearrange("p (c s) -> p c s", c=KC)
                                [:, :, dt * BL:(dt + 1) * BL])
                    psr = pg.tile([128, CW], f32, tag="gr")
                    gate_mm(psr, wh_sb["r"], hprev_src, hprev_off)
                    psz = pg.tile([128, CW], f32, tag="gz")
                    gate_mm(psz, wh_sb["z"], hprev_src, hprev_off)

                    # DVE/ACT work overlapping the psz matmuls
                    c3 = "p (c b) -> p c b"
                    tr = el.tile([128, CW], f32, tag="tr")
                    nc.vector.tensor_add(tr[:].rearrange(c3, c=KC),
                                         psr[:].rearrange(c3, c=KC), xsl("r"))
                    r = el.tile([128, CW], f32, tag="r")
                    nc.scalar.activation(r[:], tr[:], AF.Sigmoid)
                    rh = el.tile([128, CW], bf16, tag="rh")
                    nc.vector.tensor_mul(rh[:], r[:], hT[:])

                    psl = pg.tile([128, CW], f32, tag="gl")
                    gate_mm(psl, wh_sb["l"], rh, 0)

                    # overlapping the psl matmuls: z, plus w = (1-z)*h
                    tz = el.tile([128, CW], f32, tag="tz")
                    nc.vector.tensor_add(tz[:].rearrange(c3, c=KC),
                                         psz[:].rearrange(c3, c=KC), xsl("z"))
                    z = el.tile([128, CW], f32, tag="z")
                    nc.scalar.activation(z[:], tz[:], AF.Sigmoid)
                    v = el.tile([128, CW], f32, tag="v")
                    nc.vector.tensor_mul(v[:], z[:], hT[:])
                    w = el.tile([128, CW], f32, tag="w")
                    nc.vector.tensor_sub(w[:], hT[:], v[:])

                    # independent PE work to fill the h-update stall
                    if hv_prev is not None:
                        wo_pairs(bi - 1, hv_prev, dt, wo_state)

                    # tail: h_new = w + z*tanh(...), in halves so next-step
                    # matmuls (kc 0-3) start after the first half lands.
                    # The bf16 add (matmul rhs) runs first to unblock PE; a
                    # duplicate f32 add keeps the master state off the
                    # critical path without a copy.
                    tl = el.tile([128, CW], f32, tag="tl")
                    nc.vector.tensor_add(tl[:].rearrange(c3, c=KC),
                                         psl[:].rearrange(c3, c=KC), xsl("l"))
                    hh = el.tile([128, CW], f32, tag="hh")
                    u = el.tile([128, CW], f32, tag="u")
                    hTn = st.tile([128, CW], f32, tag="hT")
                    nc.scalar.activation(hh[:], tl[:], AF.Tanh)
                    nc.vector.tensor_mul(u[:], z[:], hh[:])
                    nc.vector.tensor_add(hist[:, dt * CW:(dt + 1) * CW],
                                         w[:], u[:])
                    nc.vector.tensor_add(hTn[:], w[:], u[:])
                    hT = hTn
                    hprev_src, hprev_off = hist, dt * CW

                hist_prev = hist

            # output projection of the final block
            hv_prev = hist_prev[:].rearrange("p (t c b) -> p c t b",
                                             t=BLK, c=KC)
            for dt in range(BLK):
                wo_pairs(nblk - 1, hv_prev, dt, wo_state)
            wo_flush(wo_state)

        if repeat > 1:
            # WAR fence: next iteration's phase 1 rewrites X* in DRAM.
            tc.strict_bb_all_engine_barrier()

    nc.compile()
    return nc


def get_program(T, repeat=1):
    if (T, repeat) not in _CACHE:
        _CACHE[(T, repeat)] = build_program(T, repeat)
    return _CACHE[(T, repeat)]


def make_in_maps(input, Wr, br, Wz, bz, Wl, bl, Wo, bo):
    Tt = input.shape[1]
    cols = BL * Tt
    w_common = {
        "wxr": np.ascontiguousarray(Wr[H:]), "whr": np.ascontiguousarray(Wr[:H]),
        "wxz": np.ascontiguousarray(Wz[H:]), "whz": np.ascontiguousarray(Wz[:H]),
        "wxl": np.ascontiguousarray(Wl[H:]), "whl": np.ascontiguousarray(Wl[:H]),
        "br": np.ascontiguousarray(br.reshape(H, 1)),
        "bz": np.ascontiguousarray(bz.reshape(H, 1)),
        "bl": np.ascontiguousarray(bl.reshape(H, 1)),
        "wo": np.ascontiguousarray(Wo),
        "bo": np.ascontiguousarray(bo.reshape(H, 1)),
    }
    in_maps = []
    for c in range(NCORES):
        xl = np.asarray(input[c * BL:(c + 1) * BL], dtype=np.float32)
        xTl = np.ascontiguousarray(xl.transpose(2, 1, 0).reshape(H, cols))
        in_maps.append({"xT": xTl, **w_common})
    return in_maps


def assemble_output(results, Tt):
    nblk = Tt // BLK
    outs = []
    for c in range(NCORES):
        oT = results[c]["outT"]                  # [128, nblk*JC*BLK*BL]
        o = oT.reshape(128, nblk, JC, BLK, BL)   # p, bi, j, dt, b
        o = o.transpose(4, 1, 3, 2, 0).reshape(BL, Tt, H)
        outs.append(o)
    return np.ascontiguousarray(np.concatenate(outs, axis=0))


def kernel(input, Wr, br, Wz, bz, Wl, bl, Wo, bo):
    Tt = input.shape[1]
    prog = get_program(Tt)
    in_maps = make_in_maps(input, Wr, br, Wz, bz, Wl, bl, Wo, bo)
    res = bass_utils.run_bass_kernel_spmd(prog, in_maps,
                                          core_ids=list(range(NCORES)))
    return assemble_output(res.results, Tt)

